# revision 6
# baseline (speedup 1.0000x reference)
"""CRF Viterbi decode kernel for Trainium2 (8 NeuronCores, data-parallel over batch).

Problem: emissions [70, 32768, 37] fp32, mask [70, 32768] (all ones),
start/end transitions [37], transitions [37, 37].
Output: best tag sequence per batch element, [32768, 70] int32.

Strategy per core (B_core = 4096 = 32 partition-tiles of 128 batch rows):
  Forward max-plus scan with batch on partitions and the (j_next, i_prev)
  tag-pair expansion (37*37 = 1369) on the free dim. Exact fp32 semantics,
  including the reference's associativity  w = (score + trans) + em  and
  first-index argmax tie-breaking (via reverse-index code + max-reduce).
  Backpointers stored on-chip (uint8). Backtracking uses a one-hot
  select-and-reduce per step, entirely on-chip.
"""

import os
import numpy as np

S = 70
T = 37
B = 32768
NCORES = 8
BC = B // NCORES          # 4096 batch rows per core
NT = BC // 128            # 32 partition tiles per core
G = 4                     # tiles per instruction group
NG = NT // G              # groups per core

_PROGRAM_CACHE = {}


def _build_program(s_len=S, ng=NG, g=G):
    import concourse.bass as bass
    import concourse.tile as tile
    from concourse import bacc, mybir

    f32 = mybir.dt.float32
    u8 = mybir.dt.uint8
    i32 = mybir.dt.int32
    Alu = mybir.AluOpType
    X = mybir.AxisListType.X

    nt = ng * g              # partition tiles
    bc = nt * 128            # batch rows this core
    TT2 = T * T              # 1369
    NC2 = TT2 + 3 * T        # consts packed: transflat, revi, iota, endt

    nc = bacc.Bacc()

    # em layout host-prepped: [s, group, 128, g*T]
    em_d = nc.declare_dram_parameter("em", [s_len, ng, 128, g * T], f32, isOutput=False)
    # score0 layout host-prepped: [128, nt*T]
    score0_d = nc.declare_dram_parameter("score0", [128, nt * T], f32, isOutput=False)
    consts_d = nc.declare_dram_parameter("consts", [128, NC2], f32, isOutput=False)
    s_out = s_len
    tags_d = nc.declare_dram_parameter("tags", [bc, s_out], i32, isOutput=True)

    with tile.TileContext(nc) as tc:
        with (
            tc.tile_pool(name="const", bufs=1) as cpool,
            tc.tile_pool(name="em", bufs=3) as empool,
            tc.tile_pool(name="score", bufs=1) as spool,
            tc.tile_pool(name="zbuf", bufs=1) as zpool,
            tc.tile_pool(name="wbuf", bufs=1) as wpool,
            tc.tile_pool(name="hist", bufs=1) as hpool,
            tc.tile_pool(name="bt", bufs=2) as btpool,
            tc.tile_pool(name="small", bufs=2) as smpool,
        ):
            # ---- constants (single DMA) ----
            consts = cpool.tile([128, NC2], f32)
            nc.sync.dma_start(consts[:], consts_d[:])
            transflat = consts[:, 0:TT2]
            revi = consts[:, TT2 : TT2 + T]
            iota = consts[:, TT2 + T : TT2 + 2 * T]
            endt = consts[:, TT2 + 2 * T : TT2 + 3 * T]

            # ---- persistent state ----
            hist = hpool.tile([128, (s_len - 1) * nt * T], u8)
            tags_sb = hpool.tile([128, nt * s_out], i32, tag="tags_sb")

            # score ping-pong buffers [128, nt*T]
            score_a = spool.tile([128, nt * T], f32, tag="score_a")
            score_b = spool.tile([128, nt * T], f32, tag="score_b")
            nc.sync.dma_start(score_a[:], score0_d[:])
            cur_score, nxt_score = score_a, score_b

            tf_b = (
                transflat.rearrange("p (j i) -> p j i", i=T)
                .unsqueeze(1)
                .broadcast_to([128, g, T, T])
            )
            revi_b = revi.unsqueeze(1).unsqueeze(1).broadcast_to([128, g, T, T])

            # ---- forward scan ----
            for s in range(1, s_len):
                for gi in range(ng):
                    em_t = empool.tile([128, g * T], f32, tag="em")
                    nc.sync.dma_start(em_t[:], em_d[s, gi])

                    sc3 = cur_score[:, gi * g * T : (gi + 1) * g * T].rearrange(
                        "p (tt i) -> p tt i", i=T
                    )
                    sc_b = sc3.unsqueeze(2).broadcast_to([128, g, T, T])
                    em_b = (
                        em_t[:]
                        .rearrange("p (tt j) -> p tt j", j=T)
                        .unsqueeze(3)
                        .broadcast_to([128, g, T, T])
                    )

                    zt = zpool.tile([128, g * TT2], f32, tag="z")
                    z4 = zt[:].rearrange("p (tt j i) -> p tt j i", j=T, i=T)
                    # z = score + trans   (score[b,i] + trans[i,j] at [j,i])
                    nc.vector.tensor_tensor(z4, sc_b, tf_b, Alu.add)

                    wt = wpool.tile([128, g * TT2], f32, tag="w")
                    w4 = wt[:].rearrange("p (tt j i) -> p tt j i", j=T, i=T)
                    # w = z + em[b,j]
                    nc.vector.tensor_tensor(w4, z4, em_b, Alu.add)

                    # new score = max_i w
                    ns3 = nxt_score[:, gi * g * T : (gi + 1) * g * T].rearrange(
                        "p (tt j) -> p tt j", j=T
                    )
                    nc.vector.tensor_reduce(ns3, w4, X, Alu.max)

                    # eq = (w == best)  (in place into z buffer)
                    ns_b = ns3.unsqueeze(3).broadcast_to([128, g, T, T])
                    nc.vector.tensor_tensor(z4, w4, ns_b, Alu.is_equal)

                    # code = eq * (37 - i)   (in place into w buffer)
                    nc.vector.tensor_tensor(w4, z4, revi_b, Alu.mult)

                    # hist codes = max_i code  -> uint8
                    hoff = ((s - 1) * nt + gi * g) * T
                    hslice = hist[:, hoff : hoff + g * T].rearrange(
                        "p (tt j) -> p tt j", j=T
                    )
                    nc.vector.tensor_reduce(hslice, w4, X, Alu.max)

                cur_score, nxt_score = nxt_score, cur_score

            # ---- final argmax over tags (score + end_transitions) ----
            cur = btpool.tile([128, nt], f32, tag="cur")
            endt_b = endt.unsqueeze(1).broadcast_to([128, g, T])
            revi_b2 = revi.unsqueeze(1).broadcast_to([128, g, T])
            for gi in range(ng):
                sc3 = cur_score[:, gi * g * T : (gi + 1) * g * T].rearrange(
                    "p (tt j) -> p tt j", j=T
                )
                se = smpool.tile([128, g * T], f32, tag="se")
                se3 = se[:].rearrange("p (tt j) -> p tt j", j=T)
                nc.vector.tensor_tensor(se3, sc3, endt_b, Alu.add)
                b1 = smpool.tile([128, g], f32, tag="b1")
                nc.vector.tensor_reduce(b1[:], se3, X, Alu.max)
                b1_b = b1[:].unsqueeze(2).broadcast_to([128, g, T])
                eqf = smpool.tile([128, g * T], f32, tag="eqf")
                eqf3 = eqf[:].rearrange("p (tt j) -> p tt j", j=T)
                nc.vector.tensor_tensor(eqf3, se3, b1_b, Alu.is_equal)
                nc.vector.tensor_tensor(eqf3, eqf3, revi_b2, Alu.mult)
                codef = smpool.tile([128, g], f32, tag="codef")
                nc.vector.tensor_reduce(codef[:], eqf3, X, Alu.max)
                # cur = 37 - code
                nc.vector.tensor_scalar(
                    cur[:, gi * g : (gi + 1) * g], codef[:], -1.0, float(T), Alu.mult, Alu.add
                )

            # tags column s_len-1
            tags3 = tags_sb[:].rearrange("p (tt s) -> p tt s", s=s_out)
            nc.vector.tensor_copy(tags3[:, :, s_len - 1], cur[:])

            # ---- backtracking ----
            iota_b = iota.unsqueeze(1).broadcast_to([128, nt, T])
            for s in range(s_len - 1, 0, -1):
                cur_b = cur[:].unsqueeze(2).broadcast_to([128, nt, T])
                eqb = btpool.tile([128, nt * T], f32, tag="eqb")
                eqb3 = eqb[:].rearrange("p (tt i) -> p tt i", i=T)
                nc.vector.tensor_tensor(eqb3, iota_b, cur_b, Alu.is_equal)
                hoff = (s - 1) * nt * T
                h3 = hist[:, hoff : hoff + nt * T].rearrange("p (tt i) -> p tt i", i=T)
                nc.vector.tensor_tensor(eqb3, eqb3, h3, Alu.mult)
                pcode = btpool.tile([128, nt], f32, tag="pcode")
                nc.vector.tensor_reduce(pcode[:], eqb3, X, Alu.max)
                ncur = btpool.tile([128, nt], f32, tag="cur")
                nc.vector.tensor_scalar(ncur[:], pcode[:], -1.0, float(T), Alu.mult, Alu.add)
                cur = ncur
                nc.vector.tensor_copy(tags3[:, :, s - 1], cur[:])

            # ---- output DMA ----
            nc.sync.dma_start(
                tags_d[:].rearrange("(tt p) s -> p tt s", p=128),
                tags3,
            )

    nc.finalize()
    return nc


def _host_prep(emissions, mask, start_transitions, end_transitions, transitions,
               s_len=S, ng=NG, g=G, ncores=NCORES):
    nt = ng * g
    bc = nt * 128
    em = np.ascontiguousarray(np.asarray(emissions, dtype=np.float32))
    start = np.asarray(start_transitions, dtype=np.float32)
    end = np.asarray(end_transitions, dtype=np.float32)
    trans = np.asarray(transitions, dtype=np.float32)

    score0 = (start[None, :] + em[0]).astype(np.float32)  # [B, T]

    # per-core reorders
    b_total = em.shape[1]
    em_r = em.reshape(s_len, b_total // bc, ng, g, 128, T)
    # -> [core][s, ng, 128, g*T]
    em_cores = [
        np.ascontiguousarray(em_r[:, c].transpose(0, 1, 3, 2, 4).reshape(s_len, ng, 128, g * T))
        for c in range(b_total // bc)
    ]
    s0_r = score0.reshape(b_total // bc, nt, 128, T)
    score0_cores = [
        np.ascontiguousarray(s0_r[c].transpose(1, 0, 2).reshape(128, nt * T))
        for c in range(b_total // bc)
    ]

    # consts: transflat (j-major: trans[i,j] at j*T+i), revi, iota, endt
    transflat = np.ascontiguousarray(trans.T).reshape(T * T)
    revi = (T - np.arange(T)).astype(np.float32)
    iota = np.arange(T).astype(np.float32)
    consts = np.concatenate([transflat, revi, iota, end]).astype(np.float32)
    consts = np.broadcast_to(consts[None, :], (128, consts.size)).copy()
    return em_cores, score0_cores, consts


def kernel(emissions, mask, start_transitions, end_transitions, transitions):
    mask_np = np.asarray(mask)
    if not mask_np.all():
        return _numpy_reference(
            np.asarray(emissions, np.float32), mask_np,
            np.asarray(start_transitions, np.float32),
            np.asarray(end_transitions, np.float32),
            np.asarray(transitions, np.float32),
        )

    from concourse.bass_utils import run_bass_kernel_spmd

    em_cores, score0_cores, consts = _host_prep(
        emissions, mask, start_transitions, end_transitions, transitions
    )

    key = (S, NG, G)
    if key not in _PROGRAM_CACHE:
        _PROGRAM_CACHE[key] = _build_program(S, NG, G)
    nc = _PROGRAM_CACHE[key]

    in_maps = []
    for c in range(NCORES):
        in_maps.append(
            {"em": em_cores[c], "score0": score0_cores[c], "consts": consts}
        )

    res = run_bass_kernel_spmd(nc, in_maps, list(range(NCORES)))
    tags = np.concatenate([np.asarray(r["tags"]) for r in res.results], axis=0)
    return tags.astype(np.int32)


def _numpy_reference(em, mask, start, end, trans):
    S_, B_, T_ = em.shape
    score = (start[None, :] + em[0]).astype(np.float32)
    history = np.zeros((S_ - 1, B_, T_), dtype=np.int32)
    for s in range(1, S_):
        z = score[:, :, None] + trans[None, :, :]
        ns = z + em[s][:, None, :]
        idx = np.argmax(ns, axis=1).astype(np.int32)
        best = np.max(ns, axis=1)
        m = mask[s][:, None]
        score = np.where(m, best, score)
        history[s - 1] = idx
    score = score + end[None, :]
    seq_ends = mask.astype(np.int32).sum(0) - 1
    best_last = np.argmax(score, axis=1).astype(np.int32)
    barange = np.arange(B_)
    tags = np.zeros((S_, B_), dtype=np.int32)
    tags[S_ - 1] = best_last
    cur = best_last
    for i in range(S_ - 1, 0, -1):
        prev = history[i - 1][barange, cur]
        cur = np.where(i <= seq_ends, prev, cur).astype(np.int32)
        tags[i - 1] = cur
    tpos = np.arange(S_)[:, None]
    tags = np.where(tpos <= seq_ends[None, :], tags, -1)
    return tags.T.astype(np.int32)
